# revision 1
# baseline (speedup 1.0000x reference)
"""Trainium2 Bass kernel for InterpretableMultiHeadAttention.

Reference computation (B=4, S=2048, H=1024, 16 heads, head_dim=64):
    Q = (query @ Wq.T + bq)  -> [B,16,S,64]
    K = (key_in @ Wk.T + bk) -> [B,16,S,64]
    V = value @ Wv.T + bv    -> [B,S,64]      (single shared V head)
    attn = softmax(Q K^T / 8)                  [B,16,S,S]
    avg_attn = attn.mean(heads)                [B,S,S]
    out = (avg_attn @ V) @ Wo.T + bo           [B,S,H]
    returns (out, avg_attn)

Sharding: 8 cores = 4 batches x 2 head-groups (8 heads each). Each core
computes S_partial[b,g] = sum_{h in group} softmax_h / 16 entirely on
device (projections, scores, exp+rowsum, per-row normalization folded
into a diagonal matmul that accumulates heads in PSUM for 4 heads and a
fused DVE scale-add for the other 4). Host sums the two partials per
batch to get avg_attn, then does the tiny avg_attn @ V and output
projection (~4% of total FLOPs).
"""

import numpy as np

B, S, H = 4, 2048, 1024
NUM_HEADS = 16
D = 64
HO = 512          # per-core projected width (8 heads x 64)
NIB = H // 128    # 8 input-dim blocks
NOB = HO // 128   # 4 output-dim blocks (= head pairs)
NQT = S // 128    # 16 query tiles
NPAIR = 4

_CACHE = {}


def _build_nc():
    import concourse.bacc as bacc
    import concourse.mybir as mybir
    import concourse.tile as tile

    f32 = mybir.dt.float32
    AF = mybir.ActivationFunctionType
    OP = mybir.AluOpType

    nc = bacc.Bacc("TRN2", target_bir_lowering=False, debug=False)

    xqT = nc.dram_tensor("xqT", [H, S], f32, kind="ExternalInput")
    xkT = nc.dram_tensor("xkT", [H, S], f32, kind="ExternalInput")
    wqT = nc.dram_tensor("wqT", [H, HO], f32, kind="ExternalInput")
    wkT = nc.dram_tensor("wkT", [H, HO], f32, kind="ExternalInput")
    bq_d = nc.dram_tensor("bq", [HO, 1], f32, kind="ExternalInput")
    bk_d = nc.dram_tensor("bk", [HO, 1], f32, kind="ExternalInput")
    ident = nc.dram_tensor("ident", [128, 128], f32, kind="ExternalInput")
    s_out = nc.dram_tensor("s_out", [S, S], f32, kind="ExternalOutput")

    with tile.TileContext(nc) as tc:
        with (
            tc.tile_pool(name="wp", bufs=1) as wp,
            tc.tile_pool(name="xp", bufs=2) as xp,
            tc.tile_pool(name="qk", bufs=1) as qk,
            tc.tile_pool(name="ep", bufs=4) as ep,
            tc.tile_pool(name="sp", bufs=2) as sp,
            tc.tile_pool(name="smp", bufs=2) as smp,
            tc.tile_pool(name="psA", bufs=2, space="PSUM") as psA,
            tc.tile_pool(name="psB", bufs=1, space="PSUM") as psB,
        ):
            wq_sb = wp.tile([128, NIB, HO], f32, tag="wq")
            wk_sb = wp.tile([128, NIB, HO], f32, tag="wk")
            bq_sb = wp.tile([128, NOB], f32, tag="bq")
            bk_sb = wp.tile([128, NOB], f32, tag="bk")
            id_sb = wp.tile([128, 128], f32, tag="ident")
            qt_sb = qk.tile([128, NOB, S], f32, tag="qt")
            kt_sb = qk.tile([128, NOB, S], f32, tag="kt")

            for ib in range(NIB):
                nc.sync.dma_start(wq_sb[:, ib, :], wqT[ib * 128:(ib + 1) * 128, :])
                nc.sync.dma_start(wk_sb[:, ib, :], wkT[ib * 128:(ib + 1) * 128, :])
            for ob in range(NOB):
                nc.sync.dma_start(bq_sb[:, ob:ob + 1], bq_d[ob * 128:(ob + 1) * 128, :])
                nc.sync.dma_start(bk_sb[:, ob:ob + 1], bk_d[ob * 128:(ob + 1) * 128, :])
            nc.sync.dma_start(id_sb[:], ident[:])

            # ---- projections: qt_sb[o, s] = sum_i wT[i,o] xT[i,s]  (+ bias) ----
            for xT, w_sb, b_sb, dst in (
                (xqT, wq_sb, bq_sb, qt_sb),
                (xkT, wk_sb, bk_sb, kt_sb),
            ):
                for sc_i in range(4):  # s chunks of 512
                    s0 = sc_i * 512
                    xs = xp.tile([128, NIB, 512], f32, tag="x", name=f"xs{sc_i}")
                    for ib in range(NIB):
                        nc.sync.dma_start(
                            xs[:, ib, :], xT[ib * 128:(ib + 1) * 128, s0:s0 + 512]
                        )
                    for ob in range(NOB):
                        acc = psA.tile([128, 1024], f32, tag="ps", name="acc")
                        for ib in range(NIB):
                            nc.tensor.matmul(
                                acc[:, :512],
                                w_sb[:, ib, ob * 128:(ob + 1) * 128],
                                xs[:, ib, :],
                                start=(ib == 0),
                                stop=(ib == NIB - 1),
                            )
                        nc.vector.tensor_scalar(
                            dst[:, ob, s0:s0 + 512], acc[:, :512],
                            b_sb[:, ob:ob + 1], None, OP.add,
                        )

            # ---- attention ----
            for qi in range(NQT):
                q0 = qi * 128
                sacc = psB.tile([128, S], f32, tag="sacc", name="sacc")
                sdve = sp.tile([128, S], f32, tag="sdve", name="sdve")
                for pair in range(NPAIR):
                    e0 = ep.tile([128, S], f32, tag="e", name="e0")
                    e1 = ep.tile([128, S], f32, tag="e", name="e1")
                    racc = smp.tile([128, 2, 2], f32, tag="racc", bufs=3, name="racc")
                    for ci in range(2):  # k chunks of 1024
                        k0 = ci * 1024
                        for half, e_t in ((0, e0), (1, e1)):
                            lo, hi = half * 64, (half + 1) * 64
                            sc = psA.tile([128, 1024], f32, tag="ps", name="sc")
                            for sub in range(2):
                                nc.tensor.matmul(
                                    sc[:, sub * 512:(sub + 1) * 512],
                                    qt_sb[lo:hi, pair, q0:q0 + 128],
                                    kt_sb[lo:hi, pair,
                                          k0 + sub * 512:k0 + (sub + 1) * 512],
                                    start=True,
                                    stop=True,
                                )
                            nc.scalar.activation(
                                e_t[:, k0:k0 + 1024], sc[:], AF.Exp, scale=0.125,
                                accum_out=racc[:, half, ci:ci + 1],
                            )
                    # rr = 1 / (16 * rowsum)  (the 1/16 head-mean folded in)
                    t2 = smp.tile([128, 2], f32, tag="t2", name="t2")
                    nc.vector.tensor_tensor(
                        t2[:], racc[:, :, 0], racc[:, :, 1], OP.add
                    )
                    t16 = smp.tile([128, 2], f32, tag="t16", name="t16")
                    nc.vector.tensor_scalar(t16[:], t2[:], 16.0, None, OP.mult)
                    rr = smp.tile([128, 2], f32, tag="rr", bufs=3, name="rr")
                    nc.vector.reciprocal(rr[:], t16[:])
                    # head h0 (even): PE diagonal-matmul accumulate into PSUM
                    dg = smp.tile([128, 128], f32, tag="dg", name="dg")
                    nc.vector.tensor_scalar(dg[:], id_sb[:], rr[:, 0:1], None, OP.mult)
                    for nn in range(4):
                        nc.tensor.matmul(
                            sacc[:, nn * 512:(nn + 1) * 512],
                            dg[:],
                            e0[:, nn * 512:(nn + 1) * 512],
                            start=(pair == 0),
                            stop=(pair == NPAIR - 1),
                        )
                    # head h1 (odd): DVE fused scale-add
                    if pair == 0:
                        nc.vector.tensor_scalar(
                            sdve[:], e1[:], rr[:, 1:2], None, OP.mult
                        )
                    else:
                        nc.vector.scalar_tensor_tensor(
                            sdve[:], e1[:], rr[:, 1:2], sdve[:], OP.mult, OP.add
                        )
                st = sp.tile([128, S], f32, tag="st", name="st")
                nc.vector.tensor_tensor(st[:], sacc[:], sdve[:], OP.add)
                nc.sync.dma_start(s_out[q0:q0 + 128, :], st[:])

    nc.compile()
    return nc


def _get_nc():
    if "nc" not in _CACHE:
        _CACHE["nc"] = _build_nc()
    return _CACHE["nc"]


def _run_device(query, key_in, Wq, bq, Wk, bk, trace=False):
    from concourse.bass_utils import run_bass_kernel_spmd

    nc = _get_nc()
    ident = np.eye(128, dtype=np.float32)
    in_maps = []
    for b in range(B):
        for g in range(2):
            o0, o1 = g * HO, (g + 1) * HO
            in_maps.append({
                "xqT": np.ascontiguousarray(query[b].T),
                "xkT": np.ascontiguousarray(key_in[b].T),
                "wqT": np.ascontiguousarray(Wq[o0:o1, :].T),
                "wkT": np.ascontiguousarray(Wk[o0:o1, :].T),
                "bq": np.ascontiguousarray(bq[o0:o1].reshape(HO, 1)),
                "bk": np.ascontiguousarray(bk[o0:o1].reshape(HO, 1)),
                "ident": ident,
            })
    res = run_bass_kernel_spmd(
        nc, in_maps, core_ids=list(range(8)), trace=trace,
        trace_cores=list(range(8)) if trace else None,
    )
    return res


def kernel(query, key_in, value, Wq, bq, Wk, bk, Wv, bv, Wo, bo, _trace=False):
    query = np.asarray(query, dtype=np.float32)
    key_in = np.asarray(key_in, dtype=np.float32)
    value = np.asarray(value, dtype=np.float32)
    Wq = np.asarray(Wq, dtype=np.float32)
    bq = np.asarray(bq, dtype=np.float32)
    Wk = np.asarray(Wk, dtype=np.float32)
    bk = np.asarray(bk, dtype=np.float32)
    Wv = np.asarray(Wv, dtype=np.float32)
    bv = np.asarray(bv, dtype=np.float32)
    Wo = np.asarray(Wo, dtype=np.float32)
    bo = np.asarray(bo, dtype=np.float32)

    res = _run_device(query, key_in, Wq, bq, Wk, bk, trace=_trace)
    _CACHE["last_res"] = res

    avg_attn = np.empty((B, S, S), dtype=np.float32)
    for b in range(B):
        avg_attn[b] = res.results[2 * b]["s_out"] + res.results[2 * b + 1]["s_out"]

    V = value @ Wv.T + bv                       # [B,S,64]
    attn_out = np.matmul(avg_attn, V)           # [B,S,64]
    output = np.matmul(attn_out, Wo.T) + bo     # [B,S,H]
    return output, avg_attn
